# revision 30
# baseline (speedup 1.0000x reference)
"""Additive (Bahdanau-style) attention kernel for Trainium2, 8 NeuronCores.

reference computation (per batch b of 32, T=1024 timesteps, H=1024):
    mlp_hidden = selu([enc[b,t]; hid[b]] @ W1 + b1)     # (T, H)
    scores     = mlp_hidden @ W2 + b2                   # (T, 1)
    weights    = softmax(scores over t)
    out[b]     = sum_t weights[t] * enc[b,t]            # (H,)

Distribution: data-parallel over batch, 4 batches per core, no collectives.

Per-core algorithm (v5):
  - enc is shipped from the host pre-cast to bf16; encoder tiles are plain
    HWDGE copies, even tiles on the sync queue and odd tiles on the scalar
    queue so the two DGE pipes stream in parallel from the first
    post-preamble cycle.  w1a8's first two j-blocks ride the scalar queue
    up front (needed by the first DR matmul); the rest rides gpsimd.
  - eT (fp8, h-major) is built two ways: tt0-3 of every batch (and tt4-7
    of batch 0) by PE transpose of 128x128 blocks into PSUM, and tt4-7 of
    batches 1+ by the DMA XBAR transpose (dma_start_transpose) straight
    from DRAM into a bf16 staging tile — that removes half the transpose
    work from the PE in steady state.  Either way a PSUM/SBUF copy casts
    to fp8; copies are spread over DVE / ACT / gpsimd to balance load.
  - the mlp matmul uses fp8e4 perf_mode=DoubleRow: one instruction
    contracts 256 rows (W1a host-cast to fp8 with a 2^10 scale; descale
    rides the ACT/DVE affine inputs).
  - selu = max(x,0) + min(alpha*e^x, alpha) (+const, dropped by softmax).
    exp always on ACT; the relu alternates ACT (even j) / DVE (odd j).
    For DVE j's the whole s2 is scaled by SW (host pre-scales that j's
    bias by SW and W2 column by 1/SW).
  - scores dot / context matmul have 1-wide outputs; packed 4-way into
    disjoint 32-column PE groups via tile_position (the groups execute
    concurrently), then reduced by a masked-ones PE matmul.
  - software pipeline per batch b: loads(b+1) + XBAR transposes up front;
    PE transpose group tt of batch b+1 after main-matmul group j=tt+1;
    XBAR casts late in the loop; epilogue(b-1) pieces after j=4/5/6.
  - batch 0's j0 is emitted as [tg0 DRs][own tt4-7 transposes][tg1 DRs]
    so the PE starts as soon as transpose groups tt0-3 are cast (~4 DMAs
    earlier than waiting for the whole batch).
  - softmax: exp off the reduced score rows (scores O(1), no max shift);
    host sums the 4 packed context partial rows and divides by Z.
"""

import math

import ml_dtypes
import numpy as np

import concourse.tile as tile
from concourse.masks import make_identity
from concourse import bacc, mybir
from concourse.bass_utils import run_bass_kernel_spmd

F32 = mybir.dt.float32
BF16 = mybir.dt.bfloat16
FP8 = mybir.dt.float8e4
ALU = mybir.AluOpType
ACTF = mybir.ActivationFunctionType
DR = mybir.MatmulPerfMode.DoubleRow

N_CORES = 8
B = 32
T = 1024
H = 1024
BL = B // N_CORES          # batches per core = 4
KC = H // 128              # contraction chunks = 8
JC = H // 128              # hidden-unit chunks = 8
TT = T // 128              # 128-row t-subtiles per batch = 8

SELU_LAMBDA = 1.0507009873554805
SELU_ALPHA = 1.6732632423543772
SW = 1024.0                # fp8 scale for W1a (and odd-j s2 scale)

WARMUP = 32                # junk transposes to ungate the PE clock


def build_kernel():
    nc = bacc.Bacc("TRN2", target_bir_lowering=False, debug=False,
                   num_devices=N_CORES)

    enc = nc.dram_tensor("enc", [BL, TT, 128, H], BF16, kind="ExternalInput").ap()
    w1a8 = nc.dram_tensor("w1a8", [128, JC, KC, 128], FP8,
                          kind="ExternalInput").ap()
    w2lp = nc.dram_tensor("w2lp", [128, JC, 32], BF16, kind="ExternalInput").ap()
    hbe = nc.dram_tensor("hbe", [128, JC, BL], F32, kind="ExternalInput").ap()
    hbr = nc.dram_tensor("hbr", [128, JC, BL], F32, kind="ExternalInput").ap()
    maskb = nc.dram_tensor("maskb", [128, 2], BF16, kind="ExternalInput").ap()
    wcz = nc.dram_tensor("wcz", [128, KC, 32], BF16, kind="ExternalInput").ap()
    outp4 = nc.dram_tensor("outp4", [BL, 4, 512], F32,
                           kind="ExternalOutput").ap()
    zs = nc.dram_tensor("zs", [BL, 2], F32, kind="ExternalOutput").ap()

    with tile.TileContext(nc) as tc:
        with (
            tc.tile_pool(name="consts", bufs=1) as consts,
            tc.tile_pool(name="encp", bufs=3) as encp,
            tc.tile_pool(name="etp", bufs=2) as etp,
            tc.tile_pool(name="xbp", bufs=8) as xbp,
            tc.tile_pool(name="selu", bufs=4) as selup,
            tc.tile_pool(name="score", bufs=2) as scorep,
            tc.tile_pool(name="outp", bufs=2) as outp,
            tc.tile_pool(name="psum", bufs=2, space="PSUM") as psum,
        ):
            # DMA ordering is strict-priority via queue FIFO: the 16 DMA
            # engines round-robin across QUEUES, so anything issued on a
            # second queue steals slots from the critical path.  Batch 0's
            # tt0-3 tiles (which gate the first matmul) go first on the
            # sync queue; everything else queues behind them there or on
            # the small scalar-queue stream (w1a head + tables).
            def emit_loads(b, tts=range(TT), halves=False):
                e_ts = []
                for tt in tts:
                    e_t = encp.tile([128, H], BF16, tag="e", bufs=3 * TT,
                                    name=f"e_{b}_{tt}")
                    if halves:
                        # two half-tile DMAs so the first k-chunks can
                        # transpose ~0.5us earlier during the cold start
                        nc.sync.dma_start(out=e_t[:, 0:512],
                                          in_=enc[b, tt][:, 0:512])
                        nc.sync.dma_start(out=e_t[:, 512:1024],
                                          in_=enc[b, tt][:, 512:1024])
                    else:
                        nc.sync.dma_start(out=e_t, in_=enc[b, tt])
                    e_ts.append(e_t)
                return e_ts

            def emit_xbar(b, eng=None):
                """XBAR-transpose tt4-7 of batch b straight from DRAM to
                bf16 staging tiles (h-major [p, k, t'] layout = eT's).
                Rides the sync queue between the next batch's critical
                (tt0-3) and lazy (tt4-7, context-only) tile loads — except
                batch 1's, which go on the scalar queue so they don't
                steal DMA slots from batch 0's startup-critical stream."""
                xts = []
                for tt in range(4, TT):
                    xt = xbp.tile([128, KC, 128], BF16, tag="xb", bufs=8,
                                  name=f"xb_{b}_{tt}")
                    (eng or nc.sync).dma_start_transpose(out=xt,
                                                         in_=enc[b, tt])
                    xts.append(xt)
                return xts

            e_ts0 = emit_loads(0, range(4))

            # w1a is packed on the host as [p, j-block, k, jc] so j-block
            # slices are contiguous per partition (2KB descriptors).  The
            # head (j-blocks 0-1, needed by the first DR matmul) + small
            # tables ride the scalar HWDGE queue.
            w1a_sb = consts.tile([128, JC, KC, 128], FP8)
            nc.scalar.dma_start(out=w1a_sb[:, 0:2], in_=w1a8[:, 0:2])
            hbe_sb = consts.tile([128, JC, BL], F32)
            nc.scalar.dma_start(out=hbe_sb, in_=hbe)
            hbr_sb = consts.tile([128, JC, BL], F32)
            nc.scalar.dma_start(out=hbr_sb, in_=hbr)
            w2lp_sb = consts.tile([128, JC, 32], BF16)
            nc.scalar.dma_start(out=w2lp_sb, in_=w2lp)
            maskb_sb = consts.tile([128, 2], BF16)
            nc.scalar.dma_start(out=maskb_sb, in_=maskb)
            # zero-padded context-weight stationary: zeros come from the
            # host; each epilogue overwrites only column 0 of each block.
            wcol_pad = consts.tile([128, KC, 32], BF16)
            nc.scalar.dma_start(out=wcol_pad, in_=wcz)

            e_ts0 += emit_loads(0, range(4, TT))
            assert len(e_ts0) == TT
            # w1a tail behind batch 0's tiles on sync: needed from j2 on
            nc.sync.dma_start(out=w1a_sb[:, 2:JC], in_=w1a8[:, 2:JC])

            # identity + PE warmup: keeps the TensorE activity monitor busy
            # (so the clock ungates) while the first tiles stream in.
            identity = consts.tile([128, 128], BF16)
            make_identity(nc, identity)
            one1 = consts.tile([1, 1], F32)
            nc.vector.memset(one1, 1.0)
            junk = consts.tile([128, 128], BF16)
            nc.vector.memset(junk, 0.0)
            warm_ps = psum.tile([128, 128], BF16, tag="sc", bufs=1)
            for _ in range(WARMUP):
                nc.tensor.transpose(warm_ps, junk, junk)

            def alloc_eT(b):
                return etp.tile([128, KC, T], FP8, tag="eT", bufs=2,
                                name=f"eT_{b}")

            def cast_engine(which):
                return {"v": nc.vector.tensor_copy,
                        "p": nc.gpsimd.tensor_copy}.get(which)

            def emit_cast(eT, tt, src, which):
                """fp8 cast of one transposed tt group into eT."""
                dst = eT[:, :, tt * 128:(tt + 1) * 128]
                if which == "a":
                    nc.scalar.activation(out=dst, in_=src, func=ACTF.Copy,
                                         scale=1.0)
                elif which == "split":
                    # gates the next batch's first matmul at the phase
                    # seam: split across both engines so the early-needed
                    # chunks (kk=0,1) land first on DVE while ACT does the
                    # rest.
                    nc.vector.tensor_copy(
                        out=eT[:, 0:4, tt * 128:(tt + 1) * 128],
                        in_=src[:, 0:4, :])
                    nc.scalar.activation(
                        out=eT[:, 4:8, tt * 128:(tt + 1) * 128],
                        in_=src[:, 4:8, :], func=ACTF.Copy, scale=1.0)
                else:
                    cast_engine(which)(out=dst, in_=src)

            def emit_transpose_tt(b, e_ts, eT, tt, which=None):
                # all KC h-chunks of one t-subtile -> PSUM, then cast.
                tp = psum.tile([128, KC, 128], BF16, tag="trans", bufs=2)
                for k in range(KC):
                    nc.tensor.transpose(
                        tp[:, k, :],
                        e_ts[tt][:, k * 128:(k + 1) * 128],
                        identity,
                    )
                if which is None:
                    which = "v" if tt % 2 == 0 else "a"
                emit_cast(eT, tt, tp, which)

            def emit_scores_tail(b, sc_ps):
                # stage partials in SBUF (bf16), PE-reduce per tg to
                # partition 0, exp with row-sum accumulation.
                scs = scorep.tile([128, 512], BF16, tag="scs")
                nc.scalar.activation(out=scs, in_=sc_ps, func=ACTF.Copy,
                                     scale=1.0)
                expw = scorep.tile([1, T], F32, tag="expw")
                rs2 = scorep.tile([1, 2], F32, tag="rsum2")
                for tg in range(2):
                    scr = psum.tile([1, 512], F32, tag="ctx", bufs=1)
                    nc.tensor.matmul(
                        scr,
                        lhsT=maskb_sb[:, tg:tg + 1],
                        rhs=scs,
                        start=True, stop=True,
                    )
                    nc.scalar.activation(
                        out=expw[:, tg * 512:(tg + 1) * 512], in_=scr,
                        func=ACTF.Exp, scale=1.0, accum_out=rs2[:, tg:tg + 1])
                return expw, rs2

            # ---- epilogue pieces (for batch whose phase1 has finished) --
            def epi_weights(state):
                # weights -> padded column stationary (PE transpose).
                e_ts, expw = state[:2]
                w_ps = psum.tile([128, KC, 1], F32, tag="ctx", bufs=1)
                for c in range(KC):
                    nc.tensor.transpose(
                        w_ps[:, c, :],
                        expw[0:1, c * 128:(c + 1) * 128],
                        one1,
                    )
                nc.vector.tensor_copy(out=wcol_pad[:, :, 0:1], in_=w_ps)

            def epi_context(state):
                # context[h] partials, col-group packed 4-way.
                e_ts = state[0]
                cp = psum.tile([128, 512], F32, tag="ctx", bufs=1)
                for half in range(2):
                    for tch in range(KC):
                        pos = 64 * (tch % 2) + 32 * half
                        nc.tensor.matmul(
                            cp[pos:pos + 32, :],
                            lhsT=wcol_pad[:, tch, :],
                            rhs=e_ts[tch][:, half * 512:(half + 1) * 512],
                            start=(tch < 2),
                            stop=(tch >= KC - 2),
                            tile_position=(0, pos),
                        )
                return cp

            def epi_out(b, cp, state):
                # unnormalized context partials out; the host sums the
                # col-group partial rows and divides by the softmax Z.
                rs2 = state[2]
                cps = outp.tile([128, 512], F32, tag="cps")
                nc.vector.tensor_copy(out=cps[:, 0:256], in_=cp[:, 0:256])
                nc.scalar.activation(out=cps[:, 256:512],
                                     in_=cp[:, 256:512], func=ACTF.Copy,
                                     scale=1.0)
                nc.sync.dma_start(out=zs[b:b + 1, :], in_=rs2)
                nc.sync.dma_start(out=outp4[b], in_=cps[0:97:32, :])

            def phase1(b, e_ts, eT, next_ctx, prev_ctx, first=False,
                       final=False):
                """Main pass for batch b.

                next_ctx: (e_ts, eT, xts) of batch b+1: PE transposes of
                its tt0-3 and fp8 casts of its XBAR tt4-7 are interleaved
                into this j-loop (or None).
                prev_ctx: (b-1, state) whose epilogue is interleaved.
                first: batch 0 — j0 is emitted [tg0][own tt4-7][tg1].
                """
                s2_prev = None
                epi = {}

                if final:
                    # the last batch accumulates its scores sequentially
                    # into two [1,512] rows, one per t-half: the tail can
                    # then exp straight off PSUM with no staging copy or
                    # masked reduce.
                    sc_row0 = psum.tile([1, 512], F32, tag="sc", bufs=1,
                                        name="scr0f")
                    sc_row1 = psum.tile([1, 512], F32, tag="trans",
                                        bufs=2, name="scr1f")

                    def emit_score(j, s2):
                        nc.tensor.matmul(
                            sc_row0, lhsT=w2lp_sb[:, j, 0:1],
                            rhs=s2[:, 0:512],
                            start=(j == 0), stop=(j == JC - 1))
                        nc.tensor.matmul(
                            sc_row1, lhsT=w2lp_sb[:, j, 0:1],
                            rhs=s2[:, 512:1024],
                            start=(j == 0), stop=(j == JC - 1))
                else:
                    sc_ps = psum.tile([128, 512], F32, tag="sc", bufs=1)

                    def emit_score(j, s2):
                        for tg in range(2):
                            pos = 64 * (j % 2) + 32 * tg
                            nc.tensor.matmul(
                                sc_ps[pos:pos + 32, :],
                                lhsT=w2lp_sb[:, j, :],
                                rhs=s2[:, tg * 512:(tg + 1) * 512],
                                start=(j < 2),
                                stop=(j >= JC - 2),
                                tile_position=(0, pos),
                            )

                for j in range(JC):
                    mp = psum.tile([128, T], F32, tag="mlp", bufs=2)
                    for tg in range(2):
                        for kk in range(KC // 2):
                            nc.tensor.matmul(
                                mp[:, tg * 512:(tg + 1) * 512],
                                lhsT=w1a_sb[:, j, 2 * kk:2 * kk + 2, :],
                                rhs=eT[:, 2 * kk:2 * kk + 2,
                                       tg * 512:(tg + 1) * 512],
                                start=(kk == 0),
                                stop=(kk == KC // 2 - 1),
                                perf_mode=DR,
                            )
                        if first and j == 0 and tg == 0:
                            # own tt4-7 between the two t-halves of j0 so
                            # tg0 only gates on transpose groups tt0-3
                            for tt in range(4, TT):
                                emit_transpose_tt(b, e_ts, eT, tt,
                                                  which="split" if tt == 7
                                                  else None)
                    if next_ctx is not None:
                        ne_ts, neT, nxts = next_ctx
                        if nxts is None:
                            # all-PE transposes, front-loaded so the last
                            # cast-copy lands before the next batch's
                            # first DoubleRow matmul.
                            for tt in ([j] if j < 5 else [5, 6] if j == 5
                                       else [7] if j == 6 else []):
                                emit_transpose_tt(b + 1, ne_ts, neT, tt,
                                                  which="split" if tt == 7
                                                  else None)
                        else:
                            # tt0-3 by PE; tt4-7 arrive XBAR-transposed,
                            # only their fp8 casts remain (late, once the
                            # XBAR DMAs have drained behind e0-e3).
                            if j < 4:
                                emit_transpose_tt(b + 1, ne_ts, neT, j)
                            elif j == 5:
                                emit_cast(neT, 4, nxts[0], "v")
                                emit_cast(neT, 5, nxts[1], "a")
                            elif j == 6:
                                emit_cast(neT, 6, nxts[2], "v")
                                emit_cast(neT, 7, nxts[3], "split")
                    if prev_ctx is not None:
                        pb, pstate = prev_ctx
                        if j == 4:
                            epi_weights(pstate)
                        elif j == 5:
                            epi["cp"] = epi_context(pstate)
                        elif j == 6:
                            epi_out(pb, epi["cp"], pstate)
                    if s2_prev is not None:
                        emit_score(j - 1, s2_prev)
                    e2 = selup.tile([128, T], BF16, tag="e2")
                    nc.scalar.activation(out=e2, in_=mp, func=ACTF.Exp,
                                         bias=hbe_sb[:, j, b:b + 1],
                                         scale=1.0 / SW)
                    r2 = selup.tile([128, T], BF16, tag="r2")
                    if j % 2 == 0:
                        # ACT path: r2 = relu(mp/SW + hb)
                        nc.scalar.activation(out=r2, in_=mp, func=ACTF.Relu,
                                             bias=hbr_sb[:, j, b:b + 1],
                                             scale=1.0 / SW)
                        alpha_cap = SELU_ALPHA
                    elif final and j == 5:
                        # tail drain: the same SW-scaled relu, but on ACT
                        # (scale=1, bias already SW*hb) so DVE only has
                        # the s2 fuses left after the last matmul
                        nc.scalar.activation(out=r2, in_=mp, func=ACTF.Relu,
                                             bias=hbr_sb[:, j, b:b + 1],
                                             scale=1.0)
                        alpha_cap = SELU_ALPHA * SW
                    else:
                        # DVE path, SW-scaled: r2 = max(mp + SW*hb, 0);
                        # this j's whole s2 is scaled by SW (host divides
                        # the W2 column by SW and offsets the exp bias).
                        nc.vector.tensor_scalar(
                            out=r2, in0=mp, scalar1=hbr_sb[:, j, b:b + 1],
                            scalar2=0.0, op0=ALU.add, op1=ALU.max,
                        )
                        alpha_cap = SELU_ALPHA * SW
                    # s2 = min(e2, alpha) + r2, single fused DVE op
                    s2 = selup.tile([128, T], BF16, tag="s2", bufs=4)
                    nc.vector.scalar_tensor_tensor(
                        out=s2, in0=e2, scalar=alpha_cap, in1=r2,
                        op0=ALU.min, op1=ALU.add,
                    )
                    s2_prev = s2
                emit_score(JC - 1, s2_prev)
                if final:
                    expw = scorep.tile([1, T], F32, tag="expw")
                    rs2 = scorep.tile([1, 2], F32, tag="rsum2")
                    return (e_ts, expw, rs2, sc_row0, sc_row1)
                expw, rs2 = emit_scores_tail(b, sc_ps)
                return (e_ts, expw, rs2)

            # ---------------- top-level software pipeline ----------------
            eT = alloc_eT(0)
            for tt in range(4):
                emit_transpose_tt(0, e_ts0, eT, tt)
            e_ts = e_ts0

            prev_state = None
            for b in range(0, BL - 1):
                if b + 1 >= 2:
                    e_ts_n = emit_loads(b + 1, range(4))
                    xts_n = emit_xbar(b + 1)
                    e_ts_n += emit_loads(b + 1, range(4, TT))
                else:
                    e_ts_n = emit_loads(b + 1)
                    xts_n = None
                next_ctx = (e_ts_n, alloc_eT(b + 1), xts_n)
                prev_ctx = (b - 1, prev_state) if prev_state is not None \
                    else None
                state = phase1(b, e_ts, eT, next_ctx, prev_ctx,
                               first=(b == 0))
                prev_state = state
                e_ts, eT = next_ctx[0], next_ctx[1]

            state = phase1(BL - 1, e_ts, eT, None, (BL - 2, prev_state),
                           final=True)
            # pipelined final tail: exp each half straight off its PSUM
            # score row; each half's context matmuls start as soon as its
            # weight transposes land.
            e_ts_l, expw_l, rs2_l, sc_row0, sc_row1 = state
            nc.scalar.activation(out=expw_l[:, 0:512], in_=sc_row0,
                                 func=ACTF.Exp, scale=1.0,
                                 accum_out=rs2_l[:, 0:1])
            wps_a = psum.tile([128, 4, 1], F32, tag="ctx", bufs=1)
            for c in range(4):
                nc.tensor.transpose(wps_a[:, c, :],
                                    expw_l[0:1, c * 128:(c + 1) * 128], one1)
            nc.vector.tensor_copy(out=wcol_pad[:, 0:4, 0:1], in_=wps_a)
            # context over t-chunks 0-3 only needs the tg0 weights: start
            # it while exp1 / the second weight transpose run
            cp = psum.tile([128, 512], F32, tag="ctx", bufs=1)
            for half in range(2):
                for tch in range(4):
                    pos = 64 * (tch % 2) + 32 * half
                    nc.tensor.matmul(
                        cp[pos:pos + 32, :],
                        lhsT=wcol_pad[:, tch, :],
                        rhs=e_ts_l[tch][:, half * 512:(half + 1) * 512],
                        start=(tch < 2), stop=False,
                        tile_position=(0, pos),
                    )
            nc.scalar.activation(out=expw_l[:, 512:1024], in_=sc_row1,
                                 func=ACTF.Exp, scale=1.0,
                                 accum_out=rs2_l[:, 1:2])
            wps_b = psum.tile([128, 4, 1], F32, tag="sc", bufs=1)
            for c in range(4):
                nc.tensor.transpose(
                    wps_b[:, c, :],
                    expw_l[0:1, (c + 4) * 128:(c + 5) * 128], one1)
            nc.vector.tensor_copy(out=wcol_pad[:, 4:8, 0:1], in_=wps_b)
            for half in range(2):
                for tch in range(4, KC):
                    pos = 64 * (tch % 2) + 32 * half
                    nc.tensor.matmul(
                        cp[pos:pos + 32, :],
                        lhsT=wcol_pad[:, tch, :],
                        rhs=e_ts_l[tch][:, half * 512:(half + 1) * 512],
                        start=False, stop=(tch >= KC - 2),
                        tile_position=(0, pos),
                    )
            epi_out(BL - 1, cp, state)

    nc.compile()
    return nc


_NC_CACHE = None


def _get_nc():
    global _NC_CACHE
    if _NC_CACHE is None:
        _NC_CACHE = build_kernel()
    return _NC_CACHE


def make_in_maps(encoder_outputs, hidden_state, W1, b1, W2):
    enc = np.ascontiguousarray(np.asarray(encoder_outputs, np.float32))
    hid = np.ascontiguousarray(np.asarray(hidden_state, np.float32))
    W1 = np.asarray(W1, np.float32)
    b1 = np.asarray(b1, np.float32)
    W2 = np.asarray(W2, np.float32)

    bf16 = ml_dtypes.bfloat16
    f8 = ml_dtypes.float8_e4m3
    # cast to the HW e4m3 format, but ship the bytes under the e4m3fn
    # container dtype: the PJRT path rejects the IEEE f8E4M3 HLO type
    # while accepting f8E4M3FN, and bass's input check is fuzzy across
    # the two.
    # [p, j-block, k, jc] packing: element (p, jb, k, jc) =
    # W1a[k*128+p, jb*128+jc], so each j-block slice is contiguous per
    # partition.
    w1a8 = np.ascontiguousarray(
        (W1[:H] * SW).astype(f8).reshape(KC, 128, JC, 128)
        .transpose(1, 2, 0, 3)).view(ml_dtypes.float8_e4m3fn)

    # per-j s2 scale: odd j's selu output is scaled by SW (DVE relu path)
    jscale = np.where(np.arange(JC) % 2 == 1, SW, 1.0).astype(np.float32)
    w2l = (W2[:, 0] * SELU_LAMBDA).reshape(JC, 128) / jscale[:, None]
    w2lp = np.zeros((128, JC, 32), bf16)
    w2lp[:, :, 0] = w2l.T.astype(bf16)

    # reduction masks: tg0/half0 partials live at partitions [0,32) and
    # [64,96) (real rows 0 and 64, zeros elsewhere), tg1/half1 at the
    # complement.
    m = np.zeros((128, 2), np.float32)
    m[0:32, 0] = 1.0
    m[64:96, 0] = 1.0
    m[32:64, 1] = 1.0
    m[96:128, 1] = 1.0

    # host-side hidden-state contribution: hb[b, :] = hid[b] @ W1[H:] + b1
    hb_all = hid[0] @ W1[H:] + b1                       # (B, H) f32
    ln_alpha = math.log(SELU_ALPHA)
    ln_sw = math.log(SW)

    enc_bf16 = enc.astype(bf16)

    in_maps = []
    for c in range(N_CORES):
        sl = slice(BL * c, BL * (c + 1))
        hb = hb_all[sl].reshape(BL, JC, 128).transpose(2, 1, 0)  # (128,JC,BL)
        # exp bias: hb + ln(alpha) (+ ln(SW) for odd j so e2 = SW*alpha*e^x)
        hbe = hb + ln_alpha + ln_sw * (np.arange(JC) % 2)[None, :, None]
        # relu bias: hb (ACT, even j) or SW*hb (DVE, odd j)
        hbr = hb * np.where(np.arange(JC) % 2 == 1, SW, 1.0)[None, :, None]
        in_maps.append({
            "enc": np.ascontiguousarray(enc_bf16[sl]).reshape(BL, TT, 128, H),
            "w1a8": w1a8,
            "w2lp": w2lp,
            "hbe": np.ascontiguousarray(hbe.astype(np.float32)),
            "hbr": np.ascontiguousarray(hbr.astype(np.float32)),
            "maskb": m.astype(bf16),
            "wcz": np.zeros((128, KC, 32), bf16),
        })
    return in_maps


def kernel(encoder_outputs, hidden_state, W1, b1, W2, b2):
    # b2 shifts every score equally; softmax is shift-invariant, so it is
    # deliberately unused.
    in_maps = make_in_maps(encoder_outputs, hidden_state, W1, b1, W2)
    nc = _get_nc()
    res = run_bass_kernel_spmd(nc, in_maps, core_ids=list(range(N_CORES)))
    out = np.empty((1, B, H), np.float32)
    for c in range(N_CORES):
        z = res.results[c]["zs"].sum(axis=1, keepdims=True)   # (BL, 1)
        p4 = res.results[c]["outp4"]                          # (BL, 4, 512)
        ctx = np.concatenate([p4[:, 0] + p4[:, 2],
                              p4[:, 1] + p4[:, 3]], axis=1)   # (BL, H)
        out[0, BL * c:BL * (c + 1)] = ctx / z
    return out
